# revision 1
# baseline (speedup 1.0000x reference)
"""nn_Communication kernel: data-parallel over batch B across 8 NeuronCores.

Contract: kernel(**inputs) takes FULL unsharded inputs and returns the FULL
output tuple (batch_maps, communication_masks, communication_rates), matching
reference.reference(). Sharding: batch element b -> core b (B == n_cores == 8);
no cross-batch communication exists in the math.

Self-contained: the per-batch-element math is inlined below (bilinear affine
warp, gaussian-conv confidence smoothing, top-k cumulative-sum masking).
"""
import numpy as np
import jax
import jax.numpy as jnp
from jax import lax

THRE = 0.5
SOLVER_THRE = 2.0

H = W = 384
N_AGENTS = 8
B_FULL = 8


def _warp_affine(src, theta):
    """F.affine_grid + F.grid_sample (bilinear, zeros, align_corners=False).
    src: [N,C,H,W], theta: [N,2,3] -> [N,C,H,W]"""
    N, C, Hh, Ww = src.shape
    xs = (2.0 * jnp.arange(Ww, dtype=src.dtype) + 1.0) / Ww - 1.0
    ys = (2.0 * jnp.arange(Hh, dtype=src.dtype) + 1.0) / Hh - 1.0
    gx, gy = jnp.meshgrid(xs, ys)
    base = jnp.stack([gx, gy, jnp.ones_like(gx)], axis=-1)
    coords = jnp.einsum('nij,hwj->nhwi', theta, base)
    ix = ((coords[..., 0] + 1.0) * Ww - 1.0) * 0.5
    iy = ((coords[..., 1] + 1.0) * Hh - 1.0) * 0.5
    x0 = jnp.floor(ix)
    y0 = jnp.floor(iy)
    wx1 = ix - x0; wx0 = 1.0 - wx1
    wy1 = iy - y0; wy0 = 1.0 - wy1

    def gather(yy, xx):
        valid = ((xx >= 0) & (xx <= Ww - 1) & (yy >= 0) & (yy <= Hh - 1)).astype(src.dtype)
        xi = jnp.clip(xx, 0, Ww - 1).astype(jnp.int32)
        yi = jnp.clip(yy, 0, Hh - 1).astype(jnp.int32)
        v = jax.vmap(lambda s, yv, xv: s[:, yv, xv])(src, yi, xi)
        return v * valid[:, None]

    return (gather(y0, x0) * (wy0 * wx0)[:, None]
            + gather(y0, x0 + 1.0) * (wy0 * wx1)[:, None]
            + gather(y0 + 1.0, x0) * (wy1 * wx0)[:, None]
            + gather(y0 + 1.0, x0 + 1.0) * (wy1 * wx1)[:, None])


def _single(conf, t_mat, kernel):
    """One batch element. conf: [N,C,H,W], t_mat: [N,N,2,3], kernel: [1,1,k,k]."""
    N, C, Hh, Ww = conf.shape
    ori = jax.nn.sigmoid(conf).max(axis=1, keepdims=True)
    comm = lax.conv_general_dilated(ori, kernel, (1, 1), 'SAME',
                                    dimension_numbers=('NCHW', 'OIHW', 'NCHW'))

    ego_maps = _warp_affine(comm, t_mat[0])
    flat = ego_maps.reshape(N, -1)
    rest = flat[1:]
    idx = jnp.argsort(-rest, axis=0)
    sorted_vals = jnp.take_along_axis(rest, idx, axis=0)
    cum = jnp.cumsum(jnp.concatenate([flat[:1], sorted_vals], axis=0), axis=0)
    below = cum < SOLVER_THRE
    unsort = jnp.argsort(idx, axis=0)
    mask_rest = jnp.take_along_axis(below[1:], unsort, axis=0)
    ego_mask = jnp.concatenate([below[:1], mask_rest], axis=0).astype(comm.dtype)
    ego_mask = ego_mask.reshape(N, 1, Hh, Ww)

    communication_mask = _warp_affine(ego_mask, t_mat[:, 0])
    mask_by_conf = (comm > THRE).astype(comm.dtype)
    communication_mask = mask_by_conf * communication_mask

    rate = communication_mask[1:].sum() / (Hh * Ww * (N - 1))
    mask_nodiag = communication_mask.at[0].set(1.0)
    return ori * mask_nodiag, mask_nodiag, rate


_JITTED = {}


def _get_jitted(device):
    key = id(device)
    if key not in _JITTED:
        _JITTED[key] = jax.jit(_single, device=device)
    return _JITTED[key]


def _run_sharded(conf_np, theta_np, kernel_np, devices):
    """Dispatch batch element b to devices[b] asynchronously; gather results."""
    futs = []
    for b in range(B_FULL):
        dev = devices[b % len(devices)]
        f = _get_jitted(dev)
        conf_b = jax.device_put(jnp.asarray(conf_np[b]), dev)
        th_b = jax.device_put(jnp.asarray(theta_np[b]), dev)
        k_b = jax.device_put(jnp.asarray(kernel_np), dev)
        futs.append(f(conf_b, th_b, k_b))
    maps, masks, rates = [], [], []
    for m, mk, r in futs:
        maps.append(np.asarray(m))
        masks.append(np.asarray(mk))
        rates.append(np.asarray(r))
    return np.stack(maps), np.stack(masks), np.stack(rates)


def kernel(batch_confidence_maps, record_len, pairwise_t_matrix, kernel):
    conf_np = np.asarray(batch_confidence_maps, dtype=np.float32)
    theta_np = np.asarray(pairwise_t_matrix, dtype=np.float32)
    kernel_np = np.asarray(kernel, dtype=np.float32)

    devices = None
    try:
        devs = jax.devices()
        if len(devs) >= 1 and devs[0].platform != 'cpu':
            devices = devs
    except Exception:
        devices = None

    if devices is not None:
        try:
            maps, masks, rates = _run_sharded(conf_np, theta_np, kernel_np, devices)
        except Exception:
            devices = None
    if devices is None:
        cpu = jax.devices('cpu')
        maps, masks, rates = _run_sharded(conf_np, theta_np, kernel_np, cpu)

    B, N = maps.shape[0], maps.shape[1]
    batch_maps = maps.astype(np.float32)
    communication_masks = masks.reshape(B * N, 1, H, W).astype(np.float32)
    communication_rates = np.float32(rates.mean())
    return batch_maps, communication_masks, communication_rates
